# revision 14
# baseline (speedup 1.0000x reference)
"""Trainium2 kernel for nn_Basis_Change_I_to_HW (embedding_lookup).

The reference computes out = einsum('bi,oi->bo', input_state, P) where P is
the (8128, 4096) one-hot basis-change matrix of Passage_matrix_I_to_HW with
I=64: P[base(l)+c, l*64+c] = 1 for pixel (l, c), base(l) = 63 + 127l - l(l+1)/2.

So the GEMM is really a fixed column scatter: each row of 64 contiguous input
columns [64l, 64l+64) lands at 64 contiguous output columns [base(l),
base(l)+64).  All data blocks live inside the span [63, 6112) of the 8128-wide
output; everything outside the blocks is zero.

Strategy: data-parallel over batch (512 rows per core, 8 cores).  Per core we
process 4 tiles of 128 rows: contiguous DMA-in of (128, 4096), 32 VectorE
pair-copies that place the 64 blocks into a padded SBUF tile whose gap columns
were zeroed once, then one contiguous DMA-out of the (128, 6049) span.  The
output columns outside [63, 6112) are never written: run_bass_kernel_spmd
pre-zeroes / donates zero-filled ExternalOutput buffers, so they read back 0.
"""

import numpy as np

BATCH = 4096
IN_COLS = 4096        # 64*64 pixels
OUT_COLS = 8128       # C(128, 2)
N_CORES = 8
ROWS_PER_CORE = BATCH // N_CORES   # 512
P_DIM = 128                        # SBUF partitions per tile
N_TILES = ROWS_PER_CORE // P_DIM   # 4
NBLK = 64                          # blocks per row
BLK = 64                           # columns per block


def _base(l):
    return 63 + 127 * l - l * (l + 1) // 2


SPAN_LO = _base(0)           # 63
SPAN_HI = _base(NBLK - 1) + BLK   # 6112
SPAN = SPAN_HI - SPAN_LO     # 6049


def _expected_out_idx():
    """out column for each input column p (p = l*64 + c)."""
    l = np.repeat(np.arange(64), 64)
    c = np.tile(np.arange(64), 64)
    return l * 128 - l * (l + 1) // 2 + (64 + c - l - 1)


def _build_nc(reps=1, store_mode="span"):
    """Build the per-core module.  reps > 1 repeats the whole per-core body
    back-to-back inside one NEFF (used for differential wall-clock timing).

    store_mode:
      "span" - one store per 128-row tile covering columns [63, 6112); all
               interior gaps are zeroed in SBUF and written out.
      "pair" - one store per block pair a covering [base(2a), base(2a+1)+64);
               the 31 inter-pair gaps are never written (the runtime's
               pre-zeroed output buffers supply those zeros), saving ~16% of
               write traffic at the cost of 32 stores per tile.
    """
    import concourse.mybir as mybir
    from concourse import bacc, tile
    from concourse.ap import AP

    f32 = mybir.dt.float32
    # Bacc (not plain Bass): its finalize() runs generate_event_semaphores,
    # which splits multi-semaphore waits into chains the hardware can encode
    # (1 wait per instruction, 2 on InstEventSemaphore).
    nc = bacc.Bacc()
    x = nc.dram_tensor("x", [ROWS_PER_CORE, IN_COLS], f32, kind="ExternalInput")
    y = nc.dram_tensor("y", [ROWS_PER_CORE, OUT_COLS], f32, kind="ExternalOutput")

    # Number of 128-row tiles loaded by one SWDGE DMA.  Fewer DMAs -> fewer
    # distinct completion-semaphore lanes on the kernel-tail drain (walrus
    # caps the sync-wait count per instruction).
    TILES_PER_LOAD = 2
    N_LOADS = N_TILES // TILES_PER_LOAD

    with tile.TileContext(nc) as tc:
        with (
            tc.tile_pool(name="inp", bufs=N_LOADS) as in_pool,
            tc.tile_pool(name="outp", bufs=N_TILES) as out_pool,
        ):
            for rep in range(reps):
              for h in range(N_LOADS):
                it = in_pool.tile(
                    [P_DIM, TILES_PER_LOAD * IN_COLS], f32, tag="it",
                    name=f"it{rep}_{h}",
                )
                inf = it[:]
                # x rows h*256 + t2*128 + p  ->  it[p, t2*4096 + m]
                src = AP(
                    tensor=x[:].tensor,
                    offset=h * TILES_PER_LOAD * P_DIM * IN_COLS,
                    ap=[
                        [IN_COLS, P_DIM],
                        [P_DIM * IN_COLS, TILES_PER_LOAD],
                        [1, IN_COLS],
                    ],
                )
                nc.gpsimd.dma_start(it[:], src)

                ipitch = inf.ap[0][0]
                for t2 in range(TILES_PER_LOAD):
                    t = h * TILES_PER_LOAD + t2
                    ot = out_pool.tile(
                        [P_DIM, SPAN], f32, tag="ot", name=f"ot{rep}_{t}"
                    )
                    of = ot[:]
                    opitch = of.ap[0][0]
                    # Copies first: the only instruction-level wait they need
                    # is the load-DMA semaphore (DVE instructions only encode
                    # one sync wait).  The gap memsets come after; their WAW
                    # deps on the copies collapse onto the single DVE
                    # counting semaphore.
                    for a in range(NBLK // 2):
                        l0 = 2 * a
                        s = _base(l0 + 1) - _base(l0)     # 126 - 2a (>= BLK)
                        dst = AP(
                            tensor=of.tensor,
                            offset=of.offset + (_base(l0) - SPAN_LO),
                            ap=[[opitch, P_DIM], [s, 2], [1, BLK]],
                        )
                        csrc = AP(
                            tensor=inf.tensor,
                            offset=inf.offset + t2 * IN_COLS + l0 * BLK,
                            ap=[[ipitch, P_DIM], [BLK, 2], [1, BLK]],
                        )
                        nc.vector.tensor_copy(dst, csrc)
                    # Zero the gap columns that will be stored: all of them
                    # for "span", only intra-pair gaps (even i) for "pair".
                    for i in range(NBLK - 1):
                        if store_mode == "pair" and i % 2 == 1:
                            continue
                        g0 = _base(i) + BLK - SPAN_LO
                        g1 = _base(i + 1) - SPAN_LO
                        if g1 > g0:
                            gap = AP(
                                tensor=of.tensor,
                                offset=of.offset + g0,
                                ap=[[opitch, P_DIM], [1, g1 - g0]],
                            )
                            nc.vector.memset(gap, 0.0)

                    # Store(s) on the sync HWDGE ring; completion is only
                    # awaited by the kernel-tail drain.
                    rows = y[t * P_DIM:(t + 1) * P_DIM, :]
                    if store_mode == "span":
                        nc.sync.dma_start(
                            y[t * P_DIM:(t + 1) * P_DIM, SPAN_LO:SPAN_HI], ot[:]
                        )
                    else:
                        for a in range(NBLK // 2):
                            lo = _base(2 * a)
                            hi = _base(2 * a + 1) + BLK
                            nc.sync.dma_start(
                                y[t * P_DIM:(t + 1) * P_DIM, lo:hi],
                                ot[:, lo - SPAN_LO:hi - SPAN_LO],
                            )
    nc.finalize()
    return nc


def _run_device(input_state, trace=False):
    from concourse.bass_utils import run_bass_kernel_spmd

    nc = _build_nc()
    in_maps = [
        {"x": np.ascontiguousarray(input_state[c * ROWS_PER_CORE:(c + 1) * ROWS_PER_CORE])}
        for c in range(N_CORES)
    ]
    res = run_bass_kernel_spmd(nc, in_maps, list(range(N_CORES)), trace=trace)
    out = np.concatenate([res.results[c]["y"] for c in range(N_CORES)], axis=0)
    return out, res


def _p_matches_reference(P):
    if P.shape != (OUT_COLS, IN_COLS):
        return False
    if np.count_nonzero(P) != IN_COLS:
        return False
    return bool(np.all(P[_expected_out_idx(), np.arange(IN_COLS)] == 1.0))


def kernel(input_state, passage_matrix):
    input_state = np.ascontiguousarray(np.asarray(input_state), dtype=np.float32)
    P = np.asarray(passage_matrix)
    assert input_state.shape == (BATCH, IN_COLS)

    if _p_matches_reference(P):
        out, _ = _run_device(input_state)
        return out.astype(np.float32, copy=False)

    # Fallbacks for a P that doesn't match the hardcoded reference pattern.
    rows, cols = np.nonzero(P)
    if len(rows) == len(np.unique(rows)) and np.all(P[rows, cols] == 1.0):
        out = np.zeros((BATCH, OUT_COLS), dtype=np.float32)
        out[:, rows] = input_state[:, cols]
        return out
    return (input_state @ P.T.astype(np.float32)).astype(np.float32)
